# revision 36
# baseline (speedup 1.0000x reference)
"""7x7 median blur (kornia median_blur semantics, zero padding, lower median)
for img [8, 3, 512, 512] f32, data-parallel across 8 NeuronCores.

Algorithm: exact f32 min/max selection network evaluated on image planes.
Layout per core: rows on partitions (4 blocks of 128 per channel), columns
(padded 512+6=518) on the free dim. Horizontal window shifts are free AP
offsets; the 7 vertical taps are DMA-shifted copies of the source rows.

Network (structure "C"): per column sort 7 vertical taps (16-CE optimal
sort), build shared 2-col and 3-col merges (odd-even), merge two shifted
3-col lists into the 6-col union pruned to ranks 18..25, then select
rank 25 of (42-union, 7th column) via the min-over-splits-of-max identity.
238 min/max plane ops per (channel, block) unit after DCE; exact (bit
identical to sorting) since min/max networks compute order statistics.
"""

import numpy as np

H = 512
W = 512
C = 3
B = 8
K = 7
PAD = K // 2
WT = W + 2 * PAD  # padded tile width (518)
NBLK = H // 128   # 4 row blocks per channel


# ---------------------------------------------------------------- network ---

def _build_network(reuse=True):
    """Returns (sched_ops, n_slots, final_slot).

    sched_ops: list of ('op', out_slot, (a_slot, da), (b_slot, db), lo, hi)
      where slots < 0 encode tap inputs: slot -1-k = tap k (IMG_k).
      lo..hi inclusive is the write interval in tile index space (0..517),
      tile index i corresponds to image column i-3.
    """
    ops = []      # id -> ('in', k) | (op, (a, da), (b, db))
    cache = {}

    def inp(k):
        key = ("in", k)
        if key not in cache:
            ops.append(key)
            cache[key] = len(ops) - 1
        return (cache[key], 0)

    def mk(op, a, b):
        (ia, da), (ib, db) = a, b
        if (ia, da) > (ib, db):
            (ia, da), (ib, db) = (ib, db), (ia, da)
        base = min(da, db)
        key = (op, ia, da - base, ib, db - base)
        if key not in cache:
            ops.append((op, (ia, da - base), (ib, db - base)))
            cache[key] = len(ops) - 1
        return (cache[key], base)

    def ce(a, b):
        return mk("min", a, b), mk("max", a, b)

    def oe_merge(A, Bl):
        n, m = len(A), len(Bl)
        if n == 0:
            return list(Bl)
        if m == 0:
            return list(A)
        if n == 1 and m == 1:
            lo, hi = ce(A[0], Bl[0])
            return [lo, hi]
        E = oe_merge(A[0::2], Bl[0::2])
        O = oe_merge(A[1::2], Bl[1::2])
        out = [E[0]]
        i = 0
        while i < len(O) and i + 1 < len(E):
            lo, hi = ce(O[i], E[i + 1])
            out += [lo, hi]
            i += 1
        out += O[i:]
        out += E[i + 1:]
        return out

    def shift(ws, dx):
        return [(i, d + dx) for (i, d) in ws]

    def select_rank(A, Bl, r):
        n, m = len(A), len(Bl)
        cands = []
        for i in range(max(0, r - m), min(r, n) + 1):
            j = r - i
            if i == 0:
                cands.append(Bl[j - 1])
            elif j == 0:
                cands.append(A[i - 1])
            else:
                cands.append(mk("max", A[i - 1], Bl[j - 1]))
        while len(cands) > 1:
            nxt = []
            for q in range(0, len(cands) - 1, 2):
                nxt.append(mk("min", cands[q], cands[q + 1]))
            if len(cands) % 2:
                nxt.append(cands[-1])
            cands = nxt
        return cands[0]

    # structure C
    w = [inp(k) for k in range(K)]
    sort_pairs = [(0, 6), (2, 3), (4, 5), (0, 2), (1, 4), (3, 6), (0, 1),
                  (2, 5), (3, 4), (1, 2), (4, 6), (2, 3), (4, 5), (1, 2),
                  (3, 4), (5, 6)]
    for i, j in sort_pairs:
        lo, hi = ce(w[i], w[j])
        w[i], w[j] = lo, hi
    col = w
    m2 = oe_merge(col, shift(col, 1))
    m3 = oe_merge(m2, shift(col, 2))
    n42 = oe_merge(shift(m3, -3), m3)
    out_id, out_dx = select_rank(n42, shift(col, 3), 25)
    assert out_dx == -3

    # DCE
    needed = set()
    stack = [out_id]
    while stack:
        i = stack.pop()
        if i in needed:
            continue
        needed.add(i)
        op = ops[i]
        if op[0] != "in":
            stack.append(op[1][0])
            stack.append(op[2][0])
    sched = [i for i in range(len(ops)) if i in needed]

    # valid intervals (tile idx space, inclusive), forward pass
    valid = {}
    for i in sched:
        op = ops[i]
        if op[0] == "in":
            valid[i] = (0, WT - 1)
        else:
            _, (a, da), (b, db) = op
            lo = max(valid[a][0] - da, valid[b][0] - db, 0)
            hi = min(valid[a][1] - da, valid[b][1] - db, WT - 1)
            valid[i] = (lo, hi)

    # needed write intervals, backward pass. Output cols 0..511 live at tile
    # idx 0..511 of the final node (out_dx = -3 folds the 3-col offset).
    need_iv = {out_id: (0, W - 1)}
    for i in reversed(sched):
        op = ops[i]
        if op[0] == "in" or i not in need_iv:
            continue
        lo, hi = need_iv[i]
        for (a, da) in (op[1], op[2]):
            nlo, nhi = lo + da, hi + da
            if a in need_iv:
                nlo = min(nlo, need_iv[a][0])
                nhi = max(nhi, need_iv[a][1])
            need_iv[a] = (nlo, nhi)
    for i in sched:
        if ops[i][0] != "in":
            assert need_iv[i][0] >= valid[i][0] and need_iv[i][1] <= valid[i][1], \
                (i, need_iv[i], valid[i])

    # slot allocation (linear scan over compute nodes only)
    pos = {i: p for p, i in enumerate(sched)}
    last_use = {}
    for i in sched:
        op = ops[i]
        if op[0] == "in":
            continue
        for (a, _) in (op[1], op[2]):
            last_use[a] = max(last_use.get(a, -1), pos[i])
    last_use[out_id] = len(sched) + 1  # consumed by DMA-out

    slot_of = {}
    free = []
    n_slots = 0
    _build_network.dbg = []
    # tap inputs get negative pseudo-slots
    for i in sched:
        op = ops[i]
        if op[0] == "in":
            slot_of[i] = -1 - op[1]
    emitted = []
    for p, i in enumerate(sched):
        op = ops[i]
        if op[0] == "in":
            continue
        if free and reuse:
            s = free.pop()
        else:
            s = n_slots
            n_slots += 1
        slot_of[i] = s
        o, (a, da), (b, db) = op
        lo, hi = need_iv[i]
        emitted.append((o, s, (slot_of[a], da), (slot_of[b], db), lo, hi))
        _build_network.dbg.append((i, a, b))
        # free operands whose last use is this op (dedupe: an op may read
        # the same node at two shifts — must free its slot only once)
        for a2 in {op[1][0], op[2][0]}:
            if last_use.get(a2) == p and slot_of[a2] >= 0:
                free.append(slot_of[a2])
    # dedicate a fresh slot to the final node: its only cross-engine hazard
    # is the out-DMA WAR, and the sole writer of that slot is the final op
    # itself (whose operands are DVE-written) — so no op ever needs both a
    # tap-DMA wait and an out-DMA WAR wait (the TT ISA allows 1 sync wait)
    assert emitted and emitted[-1][1] == slot_of[out_id]
    fo = emitted[-1]
    emitted[-1] = (fo[0], n_slots, fo[2], fo[3], fo[4], fo[5])
    final = n_slots
    n_slots += 1
    return emitted, n_slots, final


# ----------------------------------------------------------------- kernel ---

_CACHE = {}


def _reduce_waits(nc, mybir):
    """Transitive reduction of semaphore waits.

    Tile emits each instruction's required vector-clock ticks as waits
    without cross-proc transitivity: if X waits on DVE>=929 and the DVE
    instruction achieving tick 929 itself (transitively) waited on
    DMAHW0>=16, then X's DMAHW0>=16 wait is redundant. This walrus build
    allows only ONE sync wait per compute/DMA instruction, so drop every
    wait implied by happens-before. Sound because sem values only increase.
    """
    import bisect
    from collections import defaultdict

    f = nc.m.functions[0]
    insts = [ins for blk in f.blocks for ins in blk.instructions]

    cum = defaultdict(int)
    sem_hist = defaultdict(lambda: ([], []))  # sem -> (values, inst idx)
    bad_sems = set()
    for idx, ins in enumerate(insts):
        si = ins.sync_info
        if not si:
            continue
        for up in (si.on_update or []):
            if getattr(up, "update_mode", None) in ("sem-inc", "sem-add-imm"):
                cum[up.id] += up.update_value
                vals, idxs = sem_hist[up.id]
                vals.append(cum[up.id])
                idxs.append(idx)
            else:
                bad_sems.add(up.id)

    def achiever(sem, v):
        if sem in bad_sems:
            return None
        vals, idxs = sem_hist.get(sem, ([], []))
        i = bisect.bisect_left(vals, v)
        return idxs[i] if i < len(vals) else None

    know = [None] * len(insts)
    last_on_proc = {}
    reducible = ("InstDMACopy", "InstTensorTensor", "InstTensorCopy",
                 "InstMemset")
    still_multi = []
    for idx, ins in enumerate(insts):
        proc = getattr(ins, "bass_scheduled_proc", None)
        base = {}
        if proc is not None and proc in last_on_proc:
            base = dict(know[last_on_proc[proc]])
        si = ins.sync_info
        waits = list(si.on_wait or []) if si else []
        usable = [w for w in waits
                  if getattr(w, "wait_mode", None) == "sem-ge-imm"
                  and w.wait_reg is None and w.id not in bad_sems]
        cur = dict(base)
        kept = list(waits)
        if si and len(waits) > 1 and len(usable) == len(waits):
            wk = []
            for w in waits:
                a = achiever(w.id, w.wait_value)
                k = dict(know[a]) if (a is not None and know[a]) else {}
                k[w.id] = max(k.get(w.id, 0), w.wait_value)
                wk.append(k)
            # keep waits not implied; greedy strongest-first
            order = sorted(range(len(waits)),
                           key=lambda i: -len(wk[i]))
            keep_idx = []
            for wi in order:
                w = waits[wi]
                if cur.get(w.id, 0) >= w.wait_value:
                    continue
                keep_idx.append(wi)
                for s, v in wk[wi].items():
                    cur[s] = max(cur.get(s, 0), v)
            kept = [waits[i] for i in sorted(keep_idx)]
            if len(kept) < len(waits):
                ins.sync_info = mybir.SyncInfo(
                    on_wait=kept, on_update=list(si.on_update or []))
        else:
            for w in usable:
                a = achiever(w.id, w.wait_value)
                if a is not None and know[a]:
                    for s, v in know[a].items():
                        cur[s] = max(cur.get(s, 0), v)
                cur[w.id] = max(cur.get(w.id, 0), w.wait_value)
        if len(kept) > 1 and ins.__class__.__name__ in reducible:
            still_multi.append((ins.name, ins.__class__.__name__,
                                [(w.ant_name, w.wait_value) for w in kept]))
        # completion implies own sem post-values
        if si:
            for up in (si.on_update or []):
                if getattr(up, "update_mode", None) in ("sem-inc", "sem-add-imm"):
                    vals, idxs = sem_hist[up.id]
                    i = bisect.bisect_left(idxs, idx)
                    if i < len(idxs) and idxs[i] == idx:
                        cur[up.id] = max(cur.get(up.id, 0), vals[i])
        know[idx] = cur
        if proc is not None:
            last_on_proc[proc] = idx
    return still_multi


def _get_bass(repeat=1, G=1):
    key = ("nc", repeat, G)
    if key in _CACHE:
        return _CACHE[key]
    import sys
    for p in ("/opt/trn_rl_repo", "/root/.axon_site/_ro/trn_rl_repo"):
        if p not in sys.path:
            sys.path.append(p)
    import concourse.bass as bass
    import concourse.tile as tile
    from concourse import mybir

    emitted, n_slots, final_slot = _build_network()
    assert NBLK % G == 0

    nc = bass.Bass("TRN2", target_bir_lowering=False, debug=False)
    # input arrives zero-row-padded from the host: rows 0..2, 515..517 = 0
    img_pad = nc.dram_tensor("img_pad", [C, H + 2 * PAD, W],
                             mybir.dt.float32, kind="ExternalInput").ap()
    out = nc.dram_tensor("out", [C, H, W], mybir.dt.float32,
                         kind="ExternalOutput").ap()
    f32 = mybir.dt.float32
    slot_bufs = 2 if G == 1 else 1

    with tile.TileContext(nc) as tc:
        with tc.tile_pool(name="taps", bufs=1) as tap_pool, \
             tc.tile_pool(name="zero", bufs=1) as zero_pool, \
             tc.tile_pool(name="slots", bufs=slot_bufs) as slot_pool:
            # persistent tap tensors [128, K, G, WT] (manual double buffer):
            # pads zeroed once; all K*G vertical taps of a group arrive via
            # ONE strided DMA so every network TT depends on exactly one
            # DMA-queue sem (each instruction may carry at most ONE wait)
            taps_b = []
            for bi in range(2):
                t = tap_pool.tile([128, K, G, WT], f32, tag=f"tap{bi}")
                nc.vector.memzero(t[:, :, :, 0:PAD])
                nc.vector.memzero(t[:, :, :, W + PAD:WT])
                taps_b.append(t)
            # dummy read of the last memzero'd pad: absorbs the DVE-sem wait
            # for the pad memzeros so the first network TTs carry only their
            # single DMA-queue wait
            dummy = zero_pool.tile([128, 1], f32, tag="dummy")
            nc.vector.tensor_copy(dummy[:, :],
                                  taps_b[1][:, K - 1, G - 1, WT - 1:WT])
            scr_b = zero_pool.tile([128, 1], f32, tag="scr_b")

            # groups of G consecutive row-blocks of one channel
            groups = [(c, b0) for c in range(C)
                      for b0 in range(0, NBLK, G)] * repeat

            def dma_taps(gi):
                c, b0 = groups[gi]
                taps = taps_b[gi % 2]
                # src element (p, k, g, x) = img_pad[c, 128*(b0+g)+p+k, x]
                base = img_pad[c, 128 * b0:128 * b0 + 1, 0:W]
                src = bass.AP(tensor=base.tensor, offset=base.offset,
                              ap=[[W, 128], [W, K], [128 * W, G], [1, W]])
                nc.sync.dma_start(out=taps[:, :, :, PAD:W + PAD], in_=src)

            dma_taps(0)
            ft_hist = []
            for gi, (c, b0) in enumerate(groups):
                if gi + 1 < len(groups):
                    dma_taps(gi + 1)
                taps = taps_b[gi % 2]

                # wait-ladder: every instruction may carry at most ONE sync
                # wait, so absorb each cross-proc hazard in its own ordered
                # dummy copy. no_sync_barrier() is a scheduler-only fence
                # (free) enforcing the order.
                # rung A: absorbs this group's tap-DMA completion wait
                nc.vector.tensor_copy(scr_b[:, :], taps[:, 0, 0, PAD:PAD + 1])
                tc.no_sync_barrier()
                # rung B: pre-touch the final-slot tile so ITS buffer's WAR
                # against the out-DMA from earlier groups is absorbed here
                # (1 wait) instead of on the final network op
                ft_tile = slot_pool.tile([128, G, WT], f32,
                                         tag=f"s{final_slot}")
                nc.vector.tensor_copy(ft_tile[:, 0, 0:1],
                                      taps_b[0][:, 0, 0, 0:1])
                tc.no_sync_barrier()

                slot_tiles = [None] * n_slots
                slot_tiles[final_slot] = ft_tile
                for (o, s, (sa, da), (sb, db), lo, hi) in emitted:
                    ta = (taps[:, -1 - sa, :, :] if sa < 0
                          else slot_tiles[sa])
                    tb = (taps[:, -1 - sb, :, :] if sb < 0
                          else slot_tiles[sb])
                    if s == final_slot:
                        tdst = ft_tile
                    else:
                        tdst = slot_pool.tile([128, G, WT], f32, tag=f"s{s}")
                        slot_tiles[s] = tdst
                    op = (mybir.AluOpType.min if o == "min"
                          else mybir.AluOpType.max)
                    nc.vector.tensor_tensor(
                        out=tdst[:, :, lo:hi + 1],
                        in0=ta[:, :, lo + da:hi + 1 + da],
                        in1=tb[:, :, lo + db:hi + 1 + db],
                        op=op)

                # WAW-breaker: DVE write over the full tap interior after
                # its last read makes the DVE the last writer of the whole
                # region, so the NEXT tap DMA into this buffer carries one
                # DVE-sem wait (covering both WAR and WAW) instead of two
                nc.vector.memzero(taps[:, :, :, PAD:W + PAD])

                # single out-DMA for the whole group: dst row = 128*(b0+g)+p
                ob = out[c, 128 * b0:128 * b0 + 1, 0:W]
                odst = bass.AP(tensor=ob.tensor, offset=ob.offset,
                               ap=[[W, 128], [128 * W, G], [1, W]])
                nc.sync.dma_start(out=odst, in_=ft_tile[:, :, 0:W])
                ft_hist.append(ft_tile)

            # tail ladder: WAR-touch the last two final-slot tiles so the
            # DVE chain observes the last out-DMA queue completions; the
            # framework tail drain then needs only a single DVE wait
            for ftl in ft_hist[-2:]:
                nc.vector.memzero(ftl[:, 0, 0:1])
                tc.no_sync_barrier()

    leftover = _reduce_waits(nc, mybir)
    assert not leftover, f"multi-wait instructions remain: {leftover[:5]}"
    _CACHE[key] = nc
    return nc


def kernel(img: np.ndarray) -> np.ndarray:
    import sys
    for p in ("/opt/trn_rl_repo", "/root/.axon_site/_ro/trn_rl_repo"):
        if p not in sys.path:
            sys.path.append(p)
    from concourse.bass_utils import run_bass_kernel_spmd

    img = np.asarray(img, dtype=np.float32)
    assert img.shape == (B, C, H, W), img.shape
    nc = _get_bass()
    padded = np.pad(img, ((0, 0), (0, 0), (PAD, PAD), (0, 0)))
    in_maps = [{"img_pad": np.ascontiguousarray(padded[b])}
               for b in range(B)]
    res = run_bass_kernel_spmd(nc, in_maps, list(range(B)))
    outs = [res.results[b]["out"] for b in range(B)]
    return np.stack(outs, axis=0).astype(np.float32)


# revision 38
# speedup vs baseline: 2.6399x; 2.6399x over previous
"""7x7 median blur (kornia median_blur semantics, zero padding, lower median)
for img [8, 3, 512, 512] f32, data-parallel across 8 NeuronCores.

Algorithm: exact f32 min/max selection network evaluated on image planes.
Layout per core: rows on partitions (4 blocks of 128 per channel), columns
(padded 512+6=518) on the free dim. Horizontal window shifts are free AP
offsets; the 7 vertical taps are DMA-shifted copies of the source rows.

Network (structure "C"): per column sort 7 vertical taps (16-CE optimal
sort), build shared 2-col and 3-col merges (odd-even), merge two shifted
3-col lists into the 6-col union pruned to ranks 18..25, then select
rank 25 of (42-union, 7th column) via the min-over-splits-of-max identity.
238 min/max plane ops per (channel, block) unit after DCE; exact (bit
identical to sorting) since min/max networks compute order statistics.
"""

import numpy as np

H = 512
W = 512
C = 3
B = 8
K = 7
PAD = K // 2
WT = W + 2 * PAD  # padded tile width (518)
NBLK = H // 128   # 4 row blocks per channel


# ---------------------------------------------------------------- network ---

def _build_network(reuse=True):
    """Returns (sched_ops, n_slots, final_slot).

    sched_ops: list of ('op', out_slot, (a_slot, da), (b_slot, db), lo, hi)
      where slots < 0 encode tap inputs: slot -1-k = tap k (IMG_k).
      lo..hi inclusive is the write interval in tile index space (0..517),
      tile index i corresponds to image column i-3.
    """
    ops = []      # id -> ('in', k) | (op, (a, da), (b, db))
    cache = {}

    def inp(k):
        key = ("in", k)
        if key not in cache:
            ops.append(key)
            cache[key] = len(ops) - 1
        return (cache[key], 0)

    def mk(op, a, b):
        (ia, da), (ib, db) = a, b
        if (ia, da) > (ib, db):
            (ia, da), (ib, db) = (ib, db), (ia, da)
        base = min(da, db)
        key = (op, ia, da - base, ib, db - base)
        if key not in cache:
            ops.append((op, (ia, da - base), (ib, db - base)))
            cache[key] = len(ops) - 1
        return (cache[key], base)

    def ce(a, b):
        return mk("min", a, b), mk("max", a, b)

    def oe_merge(A, Bl):
        n, m = len(A), len(Bl)
        if n == 0:
            return list(Bl)
        if m == 0:
            return list(A)
        if n == 1 and m == 1:
            lo, hi = ce(A[0], Bl[0])
            return [lo, hi]
        E = oe_merge(A[0::2], Bl[0::2])
        O = oe_merge(A[1::2], Bl[1::2])
        out = [E[0]]
        i = 0
        while i < len(O) and i + 1 < len(E):
            lo, hi = ce(O[i], E[i + 1])
            out += [lo, hi]
            i += 1
        out += O[i:]
        out += E[i + 1:]
        return out

    def shift(ws, dx):
        return [(i, d + dx) for (i, d) in ws]

    def select_rank(A, Bl, r):
        n, m = len(A), len(Bl)
        cands = []
        for i in range(max(0, r - m), min(r, n) + 1):
            j = r - i
            if i == 0:
                cands.append(Bl[j - 1])
            elif j == 0:
                cands.append(A[i - 1])
            else:
                cands.append(mk("max", A[i - 1], Bl[j - 1]))
        while len(cands) > 1:
            nxt = []
            for q in range(0, len(cands) - 1, 2):
                nxt.append(mk("min", cands[q], cands[q + 1]))
            if len(cands) % 2:
                nxt.append(cands[-1])
            cands = nxt
        return cands[0]

    # structure C
    w = [inp(k) for k in range(K)]
    sort_pairs = [(0, 6), (2, 3), (4, 5), (0, 2), (1, 4), (3, 6), (0, 1),
                  (2, 5), (3, 4), (1, 2), (4, 6), (2, 3), (4, 5), (1, 2),
                  (3, 4), (5, 6)]
    for i, j in sort_pairs:
        lo, hi = ce(w[i], w[j])
        w[i], w[j] = lo, hi
    col = w
    m2 = oe_merge(col, shift(col, 1))
    m3 = oe_merge(m2, shift(col, 2))
    n42 = oe_merge(shift(m3, -3), m3)
    out_id, out_dx = select_rank(n42, shift(col, 3), 25)
    assert out_dx == -3

    # DCE
    needed = set()
    stack = [out_id]
    while stack:
        i = stack.pop()
        if i in needed:
            continue
        needed.add(i)
        op = ops[i]
        if op[0] != "in":
            stack.append(op[1][0])
            stack.append(op[2][0])
    sched = [i for i in range(len(ops)) if i in needed]

    # valid intervals (tile idx space, inclusive), forward pass
    valid = {}
    for i in sched:
        op = ops[i]
        if op[0] == "in":
            valid[i] = (0, WT - 1)
        else:
            _, (a, da), (b, db) = op
            lo = max(valid[a][0] - da, valid[b][0] - db, 0)
            hi = min(valid[a][1] - da, valid[b][1] - db, WT - 1)
            valid[i] = (lo, hi)

    # needed write intervals, backward pass. Output cols 0..511 live at tile
    # idx 0..511 of the final node (out_dx = -3 folds the 3-col offset).
    need_iv = {out_id: (0, W - 1)}
    for i in reversed(sched):
        op = ops[i]
        if op[0] == "in" or i not in need_iv:
            continue
        lo, hi = need_iv[i]
        for (a, da) in (op[1], op[2]):
            nlo, nhi = lo + da, hi + da
            if a in need_iv:
                nlo = min(nlo, need_iv[a][0])
                nhi = max(nhi, need_iv[a][1])
            need_iv[a] = (nlo, nhi)
    for i in sched:
        if ops[i][0] != "in":
            assert need_iv[i][0] >= valid[i][0] and need_iv[i][1] <= valid[i][1], \
                (i, need_iv[i], valid[i])

    # slot allocation (linear scan over compute nodes only)
    pos = {i: p for p, i in enumerate(sched)}
    last_use = {}
    for i in sched:
        op = ops[i]
        if op[0] == "in":
            continue
        for (a, _) in (op[1], op[2]):
            last_use[a] = max(last_use.get(a, -1), pos[i])
    last_use[out_id] = len(sched) + 1  # consumed by DMA-out

    slot_of = {}
    free = []
    n_slots = 0
    _build_network.dbg = []
    # tap inputs get negative pseudo-slots
    for i in sched:
        op = ops[i]
        if op[0] == "in":
            slot_of[i] = -1 - op[1]
    emitted = []
    for p, i in enumerate(sched):
        op = ops[i]
        if op[0] == "in":
            continue
        if free and reuse:
            s = free.pop()
        else:
            s = n_slots
            n_slots += 1
        slot_of[i] = s
        o, (a, da), (b, db) = op
        lo, hi = need_iv[i]
        emitted.append((o, s, (slot_of[a], da), (slot_of[b], db), lo, hi))
        _build_network.dbg.append((i, a, b))
        # free operands whose last use is this op (dedupe: an op may read
        # the same node at two shifts — must free its slot only once)
        for a2 in {op[1][0], op[2][0]}:
            if last_use.get(a2) == p and slot_of[a2] >= 0:
                free.append(slot_of[a2])
    # dedicate a fresh slot to the final node: its only cross-engine hazard
    # is the out-DMA WAR, and the sole writer of that slot is the final op
    # itself (whose operands are DVE-written) — so no op ever needs both a
    # tap-DMA wait and an out-DMA WAR wait (the TT ISA allows 1 sync wait)
    assert emitted and emitted[-1][1] == slot_of[out_id]
    fo = emitted[-1]
    emitted[-1] = (fo[0], n_slots, fo[2], fo[3], fo[4], fo[5])
    final = n_slots
    n_slots += 1
    return emitted, n_slots, final


# ----------------------------------------------------------------- kernel ---

_CACHE = {}


def _reduce_waits(nc, mybir):
    """Transitive reduction of semaphore waits.

    Tile emits each instruction's required vector-clock ticks as waits
    without cross-proc transitivity: if X waits on DVE>=929 and the DVE
    instruction achieving tick 929 itself (transitively) waited on
    DMAHW0>=16, then X's DMAHW0>=16 wait is redundant. This walrus build
    allows only ONE sync wait per compute/DMA instruction, so drop every
    wait implied by happens-before. Sound because sem values only increase.
    """
    import bisect
    from collections import defaultdict

    f = nc.m.functions[0]
    insts = [ins for blk in f.blocks for ins in blk.instructions]

    cum = defaultdict(int)
    sem_hist = defaultdict(lambda: ([], []))  # sem -> (values, inst idx)
    bad_sems = set()
    for idx, ins in enumerate(insts):
        si = ins.sync_info
        if not si:
            continue
        for up in (si.on_update or []):
            if getattr(up, "update_mode", None) in ("sem-inc", "sem-add-imm"):
                cum[up.id] += up.update_value
                vals, idxs = sem_hist[up.id]
                vals.append(cum[up.id])
                idxs.append(idx)
            else:
                bad_sems.add(up.id)

    def achiever(sem, v):
        if sem in bad_sems:
            return None
        vals, idxs = sem_hist.get(sem, ([], []))
        i = bisect.bisect_left(vals, v)
        return idxs[i] if i < len(vals) else None

    know = [None] * len(insts)
    last_on_proc = {}
    reducible = ("InstDMACopy", "InstTensorTensor", "InstTensorCopy",
                 "InstMemset")
    still_multi = []
    for idx, ins in enumerate(insts):
        proc = getattr(ins, "bass_scheduled_proc", None)
        base = {}
        if proc is not None and proc in last_on_proc:
            base = dict(know[last_on_proc[proc]])
        si = ins.sync_info
        waits = list(si.on_wait or []) if si else []
        usable = [w for w in waits
                  if getattr(w, "wait_mode", None) == "sem-ge-imm"
                  and w.wait_reg is None and w.id not in bad_sems]
        cur = dict(base)
        kept = list(waits)
        if si and len(waits) > 1 and len(usable) == len(waits):
            wk = []
            for w in waits:
                a = achiever(w.id, w.wait_value)
                k = dict(know[a]) if (a is not None and know[a]) else {}
                k[w.id] = max(k.get(w.id, 0), w.wait_value)
                wk.append(k)
            # keep waits not implied; greedy strongest-first
            order = sorted(range(len(waits)),
                           key=lambda i: -len(wk[i]))
            keep_idx = []
            for wi in order:
                w = waits[wi]
                if cur.get(w.id, 0) >= w.wait_value:
                    continue
                keep_idx.append(wi)
                for s, v in wk[wi].items():
                    cur[s] = max(cur.get(s, 0), v)
            kept = [waits[i] for i in sorted(keep_idx)]
            if len(kept) < len(waits):
                ins.sync_info = mybir.SyncInfo(
                    on_wait=kept, on_update=list(si.on_update or []))
        else:
            for w in usable:
                a = achiever(w.id, w.wait_value)
                if a is not None and know[a]:
                    for s, v in know[a].items():
                        cur[s] = max(cur.get(s, 0), v)
                cur[w.id] = max(cur.get(w.id, 0), w.wait_value)
        if len(kept) > 1 and ins.__class__.__name__ in reducible:
            still_multi.append((ins.name, ins.__class__.__name__,
                                [(w.ant_name, w.wait_value) for w in kept]))
        # completion implies own sem post-values
        if si:
            for up in (si.on_update or []):
                if getattr(up, "update_mode", None) in ("sem-inc", "sem-add-imm"):
                    vals, idxs = sem_hist[up.id]
                    i = bisect.bisect_left(idxs, idx)
                    if i < len(idxs) and idxs[i] == idx:
                        cur[up.id] = max(cur.get(up.id, 0), vals[i])
        know[idx] = cur
        if proc is not None:
            last_on_proc[proc] = idx
    return still_multi


def _get_bass(repeat=1, G=1):
    key = ("nc", repeat, G)
    if key in _CACHE:
        return _CACHE[key]
    import sys
    for p in ("/opt/trn_rl_repo", "/root/.axon_site/_ro/trn_rl_repo"):
        if p not in sys.path:
            sys.path.append(p)
    import concourse.bass as bass
    import concourse.tile as tile
    from concourse import mybir

    emitted, n_slots, final_slot = _build_network()
    assert NBLK % G == 0

    nc = bass.Bass("TRN2", target_bir_lowering=False, debug=False)
    # input arrives zero-row-padded from the host: rows 0..2, 515..517 = 0
    img_pad = nc.dram_tensor("img_pad", [C, H + 2 * PAD, W],
                             mybir.dt.float32, kind="ExternalInput").ap()
    out = nc.dram_tensor("out", [C, H, W], mybir.dt.float32,
                         kind="ExternalOutput").ap()
    f32 = mybir.dt.float32
    slot_bufs = 2 if G == 1 else 1

    with tile.TileContext(nc) as tc:
        with tc.tile_pool(name="taps", bufs=1) as tap_pool, \
             tc.tile_pool(name="zero", bufs=1) as zero_pool, \
             tc.tile_pool(name="slots", bufs=slot_bufs) as slot_pool:
            # persistent tap tensors [128, K, G, WT] (manual double buffer):
            # pads zeroed once; all K*G vertical taps of a group arrive via
            # ONE strided DMA so every network TT depends on exactly one
            # DMA-queue sem (each instruction may carry at most ONE wait)
            taps_b = []
            for bi in range(2):
                t = tap_pool.tile([128, K, G, WT], f32, tag=f"tap{bi}")
                nc.vector.memzero(t[:, :, :, 0:PAD])
                nc.vector.memzero(t[:, :, :, W + PAD:WT])
                taps_b.append(t)
            # dummy read of the last memzero'd pad: absorbs the DVE-sem wait
            # for the pad memzeros so the first network TTs carry only their
            # single DMA-queue wait
            dummy = zero_pool.tile([128, 1], f32, tag="dummy")
            nc.vector.tensor_copy(dummy[:, :],
                                  taps_b[1][:, K - 1, G - 1, WT - 1:WT])
            scr_b = zero_pool.tile([128, 1], f32, tag="scr_b")

            # groups of G consecutive row-blocks of one channel
            groups = [(c, b0) for c in range(C)
                      for b0 in range(0, NBLK, G)] * repeat

            def dma_taps(gi):
                c, b0 = groups[gi]
                taps = taps_b[gi % 2]
                # src element (p, k, x) = img_pad[c, 128*(b0+g)+p+k, x];
                # one DMA per g-slice (4-dim src/dst APs don't balance)
                for g in range(G):
                    base = img_pad[c, 128 * (b0 + g):128 * (b0 + g) + 1, 0:W]
                    srcap = bass.AP(tensor=base.tensor, offset=base.offset,
                                    ap=[[W, 128], [W, K], [1, W]])
                    nc.sync.dma_start(out=taps[:, :, g, PAD:W + PAD],
                                      in_=srcap)

            dma_taps(0)
            ft_hist = []
            for gi, (c, b0) in enumerate(groups):
                if gi + 1 < len(groups):
                    dma_taps(gi + 1)
                taps = taps_b[gi % 2]

                # fan-in: one tiny copy per g-slice absorbs that slice's
                # tap-DMA queue wait (each instruction carries at most ONE
                # sync wait); the fences order them so later TTs reading
                # both slices need no waits of their own
                for g in range(G):
                    nc.vector.tensor_copy(scr_b[:, :],
                                          taps[:, 0, g, PAD:PAD + 1])
                    tc.no_sync_barrier()
                ft_tile = slot_pool.tile([128, G, WT], f32,
                                         tag=f"s{final_slot}")

                slot_tiles = [None] * n_slots
                slot_tiles[final_slot] = ft_tile
                for (o, s, (sa, da), (sb, db), lo, hi) in emitted:
                    ta = (taps[:, -1 - sa, :, :] if sa < 0
                          else slot_tiles[sa])
                    tb = (taps[:, -1 - sb, :, :] if sb < 0
                          else slot_tiles[sb])
                    if s == final_slot:
                        tdst = ft_tile
                    else:
                        tdst = slot_pool.tile([128, G, WT], f32, tag=f"s{s}")
                        slot_tiles[s] = tdst
                    op = (mybir.AluOpType.min if o == "min"
                          else mybir.AluOpType.max)
                    nc.vector.tensor_tensor(
                        out=tdst[:, :, lo:hi + 1],
                        in0=ta[:, :, lo + da:hi + 1 + da],
                        in1=tb[:, :, lo + db:hi + 1 + db],
                        op=op)

                # single out-DMA for the whole group: dst row = 128*(b0+g)+p
                ob = out[c, 128 * b0:128 * b0 + 1, 0:W]
                odst = bass.AP(tensor=ob.tensor, offset=ob.offset,
                               ap=[[W, 128], [128 * W, G], [1, W]])
                nc.sync.dma_start(out=odst, in_=ft_tile[:, :, 0:W])
                ft_hist.append(ft_tile)

            # tail ladder: WAR-touch the last two final-slot tiles so the
            # DVE chain observes the last out-DMA queue completions; the
            # framework tail drain then needs only a single DVE wait
            for ftl in ft_hist[-2:]:
                nc.vector.memzero(ftl[:, 0, 0:1])
                tc.no_sync_barrier()

    leftover = _reduce_waits(nc, mybir)
    assert not leftover, f"multi-wait instructions remain: {leftover[:5]}"
    _CACHE[key] = nc
    return nc


def kernel(img: np.ndarray) -> np.ndarray:
    import sys
    for p in ("/opt/trn_rl_repo", "/root/.axon_site/_ro/trn_rl_repo"):
        if p not in sys.path:
            sys.path.append(p)
    from concourse.bass_utils import run_bass_kernel_spmd

    img = np.asarray(img, dtype=np.float32)
    assert img.shape == (B, C, H, W), img.shape
    nc = _get_bass()
    padded = np.pad(img, ((0, 0), (0, 0), (PAD, PAD), (0, 0)))
    in_maps = [{"img_pad": np.ascontiguousarray(padded[b])}
               for b in range(B)]
    res = run_bass_kernel_spmd(nc, in_maps, list(range(B)))
    outs = [res.results[b]["out"] for b in range(B)]
    return np.stack(outs, axis=0).astype(np.float32)
